# revision 66
# baseline (speedup 1.0000x reference)
"""AthenaSA sliding-window attention layer on 8 TRN2 NeuronCores.

Sharding: sequence-parallel. 8 cores = 2 batches x 4 sequence chunks of 1024
tokens. Each core recomputes k/v for a 512-token halo (zero-padded for the
first chunk), so there are NO collectives — the kernel is embarrassingly
parallel and each core runs an identical Bass graph on different data.

Per-core pipeline (projections fp8-e4m3 with DoubleRow perf mode = 2x PE
throughput, attention bf16, accumulation f32):
  emb8 [128, 16, 1536] fp8 (host-pre-packed k-tile-pair layout) -> RMSNorm
  stats via fp8 ones-matmul partition reduction -> QK projections in
  transposed layout (q^T, k^T = [dk, tokens]) via fp8 DoubleRow + RoPE
  (partition-shifted reads) -> V projection in natural layout (fp8 DR) ->
  banded sliding-window attention with transposed scores (bf16), softmax
  denominators via all-ones stationary matmul -> fp8 attention output ->
  fp8 DoubleRow out-projection back to natural layout + residual.
"""
import math
import os
import sys

sys.path.insert(0, "/opt/trn_rl_repo")

import numpy as np
import ml_dtypes

import concourse.bass as bass
import concourse.bacc as bacc
import concourse.mybir as mybir
from concourse import tile
from concourse import bass_utils
from contextlib import ExitStack

BF16 = ml_dtypes.bfloat16
E4M3 = ml_dtypes.float8_e4m3

B, S, E = 2, 4096, 2048
H, HKV, DK, DV = 16, 4, 128, 128
WIN = 512
EPS = 1e-5
TOWN, TALL, HALO = 1024, 1536, 512
NE = E // 128            # 16 e-tiles
NP = NE // 2             # 8 e-tile pairs (DoubleRow)
NB = 2                   # window blocks per core
NQC = 4                  # query tiles of 128 per block
NCH = 5                  # key chunks of 128 per 640-window
GQ = H // HKV

f32 = mybir.dt.float32
bf = mybir.dt.bfloat16
f8 = mybir.dt.float8e4
AF = mybir.ActivationFunctionType
DR = mybir.MatmulPerfMode.DoubleRow
ALU = mybir.AluOpType

# fp8 weights are stored pre-scaled by WSCALE (power of 2) to sit in
# e4m3's normal range (raw values have sigma ~1/sqrt(E) = 0.022, partly
# denormal in e4m3). Compensated downstream: q/k via host cos/sin buffers,
# v via the rts copy scale, out-projection via the residual-add scale.
WSCALE = 32.0
# fp8 probs scale (folded into the host mask values): keeps exp(score)
# under e4m3 max (240) while small probs stay above the denormal flush.
# Cancels exactly in the softmax ratio (numerator and denominator both
# carry it).
PSCALE = 1.0 / 8.0


def build(tc, d):
    nc = tc.nc

    with ExitStack() as stage_all:
        stage_all.enter_context(
            nc.allow_low_precision(reason="fp8/bf16 compute path by design"))
        const_pool = stage_all.enter_context(tc.tile_pool(name="const", bufs=1))
        ones = const_pool.tile([128, 128], bf)
        nc.gpsimd.memset(ones[:], 1.0)
        ones8 = const_pool.tile([128, 2, 128], f8)
        nc.gpsimd.memset(ones8[:], 1.0)
        epsb = const_pool.tile([128, 1], f32)
        nc.gpsimd.memset(epsb[:], EPS)
        epsw = const_pool.tile([128, 1], f32)
        nc.gpsimd.memset(epsw[:], EPS * WSCALE * WSCALE)
        # exp bias: probs = exp(score + ln(PSCALE)) = exp(score)*PSCALE
        lnps = const_pool.tile([128, 1], f32)
        nc.gpsimd.memset(lnps[:], math.log(PSCALE))

        # manually-scoped pools; LIFO open/close order
        acat_cm = tc.tile_pool(name="acat", bufs=H // 2)      # ..D
        acat_pool = acat_cm.__enter__()
        wo_cm = tc.tile_pool(name="wo", bufs=1)               # ..D
        wo_pool = wo_cm.__enter__()
        emb_cm = tc.tile_pool(name="embown", bufs=1)          # ..D
        emb_pool = emb_cm.__enter__()
        kT_cm = tc.tile_pool(name="kT", bufs=HKV)             # ..C
        kT_pool = kT_cm.__enter__()
        v_cm = tc.tile_pool(name="v", bufs=1)                 # ..C
        v_pool = v_cm.__enter__()
        qT_cm = tc.tile_pool(name="qT", bufs=H // 2)          # ..C
        qT_pool = qT_cm.__enter__()
        emb8_cm = tc.tile_pool(name="emb8", bufs=1)           # ..B2
        emb8_pool = emb8_cm.__enter__()
        wkv_cm = tc.tile_pool(name="wkv", bufs=1)             # ..B1
        wkv_pool = wkv_cm.__enter__()
        rb = const_pool.tile([128, TALL], bf)                 # 1/rms, all rows
        rts = const_pool.tile([128, 12], f32)                 # 1/rms per token-tile

        # emb8: whole residual-stream chunk in fp8, token-chunked
        # k-tile-pair layout [128, 3(chunk of 512 tok), NE, 512].
        # Split DMAs so several queues pull concurrently.
        # DMA order matters: chunk c0 feeds the first Gram/V matmuls — fan it
        # across every DMA queue; wv8 is needed ~2us in, before c1/c2.
        emb8 = emb8_pool.tile([128, 3, NE, 512], f8)
        for e in range(NE):
            nc.sync.dma_start(emb8[:, 0, e, :], d["emb8"][:, 0, e, :])
        ident = const_pool.tile([128, 128], bf)
        nc.sync.dma_start(ident[:], d["ident"][:])
        wv8 = wkv_pool.tile([128, NE, HKV * DV], f8)
        for qtr in range(4):
            nc.sync.dma_start(wv8[:, 4 * qtr:4 * (qtr + 1), :],
                              d["wv8"][:, 4 * qtr:4 * (qtr + 1), :])
        for c in range(1, 3):
            for qtr in range(4):
                nc.sync.dma_start(emb8[:, c, 4 * qtr:4 * (qtr + 1), :],
                                  d["emb8"][:, c, 4 * qtr:4 * (qtr + 1), :])
        wk8 = wkv_pool.tile([128, NE, HKV * DK], f8)
        nc.sync.dma_start(wk8[:], d["wk8"][:])

        def embsl(pe, off, w):
            """emb8 [128, 2(e pair), w] AP at global token offset off."""
            c, o = divmod(off, 512)
            assert o + w <= 512
            return emb8[:, c, 2 * pe:2 * pe + 2, o:o + w]

        # ---------------- Stage B: V, K^T, Q^T projections ----------------
        # One shared PSUM region for the whole stage, 1-bank [128,512] tiles,
        # so K and Q pipelines overlap without a pool-close drain barrier.
        kT = []   # per kv head: [128(dk), TALL] bf16, rope'd
        qT = []
        with ExitStack() as sb1:
            gi_pool = sb1.enter_context(tc.tile_pool(name="gi", bufs=3))
            r_pool = sb1.enter_context(tc.tile_pool(name="rms", bufs=1))
            cs_pool = sb1.enter_context(tc.tile_pool(name="cosk", bufs=1))
            tmp_pool = sb1.enter_context(tc.tile_pool(name="ropetmp", bufs=2))
            wq_pool = sb1.enter_context(tc.tile_pool(name="wq", bufs=4))
            tmpq_pool = sb1.enter_context(tc.tile_pool(name="ropetmpq", bufs=2))

            cosk = cs_pool.tile([128, TALL], bf)
            sink = cs_pool.tile([128, TALL], bf)
            nc.sync.dma_start(cosk[:], d["coskT"][:, :])
            nc.sync.dma_start(sink[:], d["sinkT"][:, :])
            cosq = cs_pool.tile([128, TOWN], bf)
            sinq = cs_pool.tile([128, TOWN], bf)
            nc.sync.dma_start(cosq[:], d["cosqT"][:, :])
            nc.sync.dma_start(sinq[:], d["sinqT"][:, :])

            # ---- RMSNorm stats + V projection, pipelined per 512-token
            # chunk so PE work tracks the arriving emb8 DMAs. ssq per token
            # via PE Gram diagonals: G_t = emb_t^T emb_t (fp8 DR), GI_t =
            # G_t * I (DVE); rts (v-scale) via free-reduce(GI_t) and a
            # per-chunk sqrt/recip so V drains without waiting on rb.
            rts_raw = r_pool.tile([128, 12], f32)
            s_rt = r_pool.tile([128, 12], f32)
            gis = []
            v_all = v_pool.tile([128, 12, HKV * DV], f8)
            with ExitStack() as sa_ps:
                g_psum = sa_ps.enter_context(
                    tc.tile_pool(name="g_ps", bufs=2, space="PSUM"))
                ssq_psum = sa_ps.enter_context(
                    tc.tile_pool(name="ssq_ps", bufs=1, space="PSUM"))
                vps_pool = sa_ps.enter_context(
                    tc.tile_pool(name="v_ps", bufs=3, space="PSUM"))
                ssq = ssq_psum.tile([128, TALL], f32)  # 3 banks

                for c in range(3):
                    for t in range(4 * c, 4 * c + 4):
                        # pad G tiles to a full PSUM bank so accumulation
                        # groups of different t never share a bank
                        # (interleaved-start hazard)
                        g = g_psum.tile([128, 512], f32)
                        for pe in range(NP):
                            nc.tensor.matmul(
                                g[:, 0:128], embsl(pe, t * 128, 128),
                                embsl(pe, t * 128, 128),
                                start=(pe == 0), stop=(pe == NP - 1),
                                perf_mode=DR)
                        gi = gi_pool.tile([128, 128], bf)
                        nc.vector.tensor_mul(gi[:], g[:, 0:128], ident[:])
                        nc.vector.tensor_reduce(
                            rts_raw[:, t:t + 1], gi[:],
                            axis=mybir.AxisListType.X, op=ALU.add)
                        gis.append(gi)
                    # rts = 1/sqrt(ssq/E+eps)/WSCALE for this chunk's tiles:
                    # sqrt(WSCALE^2*(ssq/E + eps)) then plain reciprocal
                    csl = slice(4 * c, 4 * c + 4)
                    nc.scalar.activation(s_rt[:, csl], rts_raw[:, csl],
                                         AF.Sqrt, bias=epsw[:],
                                         scale=WSCALE * WSCALE / E)
                    nc.vector.reciprocal_approx_fast(rts[:, csl], s_rt[:, csl])
                    # V for this chunk's 4 token tiles; all 12 live in ONE
                    # fp8 tile so attention can take [128, 2(key-tile), 128]
                    # DoubleRow slices across tile pairs.
                    for t in range(4 * c, 4 * c + 4):
                        vps = vps_pool.tile([128, HKV * DV], f32)  # 1 bank
                        for j in range(2):
                            for pe in range(NP):
                                nc.tensor.matmul(
                                    vps[:, j * 256:(j + 1) * 256],
                                    embsl(pe, t * 128, 128),
                                    wv8[:, 2 * pe:2 * pe + 2,
                                        j * 256:(j + 1) * 256],
                                    start=(pe == 0), stop=(pe == NP - 1),
                                    perf_mode=DR)
                        nc.vector.tensor_scalar_mul(v_all[:, t, :], vps[:],
                                                    rts[:, t:t + 1])

                # rb = 1/sqrt(ssq/E + eps), all rows identical, via
                # ssq row-broadcast = ones^T @ GI_t (single-instruction
                # groups into ssq regions are sequential-safe). Only the
                # k/q rope factors consume rb.
                for t in range(12):
                    nc.tensor.matmul(ssq[:, t * 128:(t + 1) * 128], ones[:],
                                     gis[t][:], start=True, stop=True)
                s_sb = r_pool.tile([128, TALL], f32)
                nc.scalar.activation(s_sb[:], ssq[:], AF.Sqrt,
                                     bias=epsb[:], scale=1.0 / E)
                nc.vector.reciprocal_approx_fast(s_sb[:], s_sb[:])
                nc.vector.tensor_copy(rb[:], s_sb[:])        # cast -> bf16

            nc.vector.tensor_mul(cosk[:], cosk[:], rb[:])
            nc.vector.tensor_mul(sink[:], sink[:], rb[:])
            nc.vector.tensor_mul(cosq[:], cosq[:], rb[:, HALO:])
            nc.vector.tensor_mul(sinq[:], sinq[:], rb[:, HALO:])

            with tc.tile_pool(name="q_ps", bufs=4, space="PSUM") as qps_pool:
                # prefetch the first q-heads' weights so the Q pipeline
                # never waits on a cold 256KB DMA; split each across 2 queues
                def load_wqh(h):
                    wqh = wq_pool.tile([128, NE, DK], f8, name="wqh")
                    for hf in range(2):
                        nc.sync.dma_start(wqh[:, 8 * hf:8 * (hf + 1), :],
                                          d["wq8"][h][:, 8 * hf:8 * (hf + 1), :])
                    return wqh
                wqh_pre = [load_wqh(h) for h in range(4)]

                kps_cm = tc.tile_pool(name="k_ps", bufs=4, space="PSUM")
                kps_pool = kps_cm.__enter__()
                for hk in range(HKV):
                    kps = [kps_pool.tile([128, 512], f32, name="kps")
                           for _ in range(3)]
                    for s3 in range(3):
                        for jj in range(2):
                            for pe in range(NP):
                                nc.tensor.matmul(
                                    kps[s3][:, jj * 256:(jj + 1) * 256],
                                    wk8[:, 2 * pe:2 * pe + 2,
                                        hk * DK:(hk + 1) * DK],
                                    embsl(pe, s3 * 512 + jj * 256, 256),
                                    start=(pe == 0), stop=(pe == NP - 1),
                                    perf_mode=DR)
                    # rope: ko = cos*kraw + sin*swap(kraw). cos-term multiplies
                    # PSUM directly on DVE; the half-swap copies ride on ACT
                    # (only ACT allows the cross-partition offset).
                    ksw = tmp_pool.tile([128, TALL], bf)
                    t1 = tmp_pool.tile([128, TALL], bf)
                    ko = kT_pool.tile([128, TALL], bf, name="ko")
                    for s3 in range(3):
                        sl = slice(s3 * 512, (s3 + 1) * 512)
                        nc.scalar.copy(ksw[0:64, sl], kps[s3][64:128, :])
                        nc.scalar.copy(ksw[64:128, sl], kps[s3][0:64, :])
                        nc.vector.tensor_mul(t1[:, sl], kps[s3][:], cosk[:, sl])
                    nc.vector.tensor_mul(ko[:], ksw[:], sink[:])
                    nc.vector.tensor_add(ko[:], ko[:], t1[:])
                    kT.append(ko)
                kps_cm.__exit__(None, None, None)

                # ---------------- Q^T projection ----------------
                # wq comes host-permuted per-head [H, 128, NE, DK] so a
                # head's weights DMA contiguously; ~4 heads stay resident.
                for h in range(H):
                    hh = h % 2
                    if hh == 0:
                        qpair = qT_pool.tile([128, 2 * TOWN], bf, name="qpair")
                        qT.append(qpair)
                    wqh = wqh_pre[h] if h < 4 else load_wqh(h)
                    qps = [qps_pool.tile([128, 512], f32, name="qps")
                           for _ in range(2)]
                    for s2 in range(2):
                        for jj in range(2):
                            for pe in range(NP):
                                nc.tensor.matmul(
                                    qps[s2][:, jj * 256:(jj + 1) * 256],
                                    wqh[:, 2 * pe:2 * pe + 2, :],
                                    embsl(pe, HALO + s2 * 512 + jj * 256, 256),
                                    start=(pe == 0), stop=(pe == NP - 1),
                                    perf_mode=DR)
                    qsw = tmpq_pool.tile([128, TOWN], bf)
                    t1 = tmpq_pool.tile([128, TOWN], bf, name="t1q")
                    for s2 in range(2):
                        sl = slice(s2 * 512, (s2 + 1) * 512)
                        nc.scalar.copy(qsw[0:64, sl], qps[s2][64:128, :])
                        nc.scalar.copy(qsw[64:128, sl], qps[s2][0:64, :])
                        nc.vector.tensor_mul(t1[:, sl], qps[s2][:], cosq[:, sl])
                    # interleaved pair layout: columns (qtile, head, q)
                    qo = qpair.rearrange(
                        "p (t g q) -> p t g q", g=2, q=128)[:, :, hh, :]
                    nc.vector.tensor_mul(qo, qsw[:], sinq[:])
                    nc.vector.tensor_add(qo, qo, t1[:])
        wkv_cm.__exit__(None, None, None)
        emb8_cm.__exit__(None, None, None)

        # ---------------- Stage C: attention ----------------
        acat = []
        for p in range(H // 2):
            acat.append(acat_pool.tile([128, 2 * TOWN], f8, name="acat"))

        # prefetch the out-projection weights + residual during attention
        wo8 = wo_pool.tile([128, H, E], f8)
        for c in range(4):
            nc.sync.dma_start(wo8[:, 4 * c:4 * (c + 1), :],
                              d["wo8"][:, 4 * c:4 * (c + 1), :])
        emb_own = emb_pool.tile([128, 3, E], bf)
        for t in range(3):
            nc.sync.dma_start(emb_own[:, t, :],
                              d["emb_own"][t * 128:(t + 1) * 128, :])

        with ExitStack() as sc_stage:
            mask_pool = sc_stage.enter_context(
                tc.tile_pool(name="mask", bufs=NB * NQC))
            probs_pool = sc_stage.enter_context(tc.tile_pool(name="probs", bufs=6))
            rec_pool = sc_stage.enter_context(tc.tile_pool(name="rec", bufs=4))
            out_pool = sc_stage.enter_context(tc.tile_pool(name="outsb", bufs=3))
            scps_pool = sc_stage.enter_context(
                tc.tile_pool(name="sc_ps", bufs=2, space="PSUM"))
            dnot_pool = sc_stage.enter_context(
                tc.tile_pool(name="dnot_ps", bufs=2, space="PSUM"))

            masks = {}
            for blk in range(NB):
                for qc in range(NQC):
                    m = mask_pool.tile([128, NCH * 256], f8, name="m")
                    nc.sync.dma_start(
                        m[:], d["maskT"][blk, qc].rearrange("k c g q -> k (c g q)"))
                    masks[(blk, qc)] = m

            def emit_reduce(probs, blk, qc, p):
                """dn/rec/attention-out/acat for one (tile, head-pair)."""
                t = 4 * blk + qc
                kv = p // 2
                # denominator (cols 0:256) + attention-out (256:512) in one
                # shared psum bank; fp8 DoubleRow over key-tile pairs, plain
                # fp8 matmul for the odd 5th chunk.
                dnot = dnot_pool.tile([128, 512], f32)
                dn = dnot[:, 0:256]
                pr5 = probs[:].rearrange("k (c q) -> k c q", q=256)
                nc.tensor.matmul(dn, ones8[:], pr5[:, 0:2, :],
                                 start=True, stop=False, perf_mode=DR)
                nc.tensor.matmul(dn, ones8[:], pr5[:, 2:4, :],
                                 start=False, stop=False, perf_mode=DR)
                nc.tensor.matmul(dn, ones8[:, 0, :],
                                 probs[:, 4 * 256:5 * 256],
                                 start=False, stop=True)
                rec = rec_pool.tile([128, 256], f32)
                nc.vector.reciprocal_approx_fast(rec[:], dn)
                otp = dnot[:, 256:512]
                for ch in range(0, 4, 2):
                    nc.tensor.matmul(
                        otp,
                        v_all[:, 4 * blk + qc + ch:4 * blk + qc + ch + 2,
                              kv * DV:(kv + 1) * DV],
                        pr5[:, ch:ch + 2, :],
                        start=(ch == 0), stop=False, perf_mode=DR)
                nc.tensor.matmul(
                    otp,
                    v_all[:, 4 * blk + qc + 4, kv * DV:(kv + 1) * DV],
                    probs[:, 4 * 256:5 * 256],
                    start=False, stop=True)
                nc.vector.tensor_mul(acat[p][:, t * 256:(t + 1) * 256],
                                     otp, rec[:])

            def emit_outproj(t):
                """Out projection + residual for q-tile t, interleaved into
                the attention stream right after the tile's last head pair.
                The accumulator shares the dnot psum ring (same shape/tag)."""
                if 1 <= t <= 5:
                    # slot free after tile t-1's adds; stream tile t+2
                    nc.sync.dma_start(
                        emb_own[:, (t + 2) % 3, :],
                        d["emb_own"][(t + 2) * 128:(t + 3) * 128, :])
                out_sb = out_pool.tile([128, E], bf)
                for j in range(4):
                    op = dnot_pool.tile([128, 512], f32, name="dnot")
                    for j2 in range(2):
                        for p in range(H // 2):
                            lhs = acat[p].rearrange(
                                "p (t g q) -> p t g q", g=2, q=128)[:, t, :, :]
                            nc.tensor.matmul(
                                op[:, j2 * 256:(j2 + 1) * 256],
                                lhs,
                                wo8[:, 2 * p:2 * p + 2,
                                    j * 512 + j2 * 256:j * 512 + (j2 + 1) * 256],
                                start=(p == 0), stop=(p == H // 2 - 1),
                                perf_mode=DR)
                    nc.vector.scalar_tensor_tensor(
                        out_sb[:, j * 512:(j + 1) * 512],
                        op[:], 1.0 / WSCALE,
                        emb_own[:, t % 3, j * 512:(j + 1) * 512],
                        ALU.mult, ALU.add)
                    # per-slice output DMA overlaps the remaining matmuls
                    nc.sync.dma_start(
                        d["out"][t * 128:(t + 1) * 128,
                                 j * 512:(j + 1) * 512],
                        out_sb[:, j * 512:(j + 1) * 512])

            # Software-pipelined by one (tile, head-pair) step: the PE queue
            # alternates scores_i / reduce_{i-1}, so the reduce matmuls never
            # sit behind a wait on their own iteration's exp+mask chain.
            pending = None
            for blk in range(NB):
                for qc in range(NQC):
                    w0 = 512 * blk + 128 * qc     # key window start (local)
                    t = 4 * blk + qc              # own q-tile index
                    # chunks 1..3 are fully in-window except at the start of
                    # the whole sequence (k<0, zero-pad halo; only blk0 and
                    # only chunks ch <= 3-qc) — mask those spans in place,
                    # plus the triangular boundary chunks 0 and 4.
                    w1 = (4 - qc) * 256 if blk == 0 else 256
                    for p in range(H // 2):
                        kv = p // 2                # 2 pairs per kv head
                        scp = scps_pool.tile([128, NCH * 256], f32)
                        for ch in range(NCH):
                            nc.tensor.matmul(
                                scp[:, ch * 256:(ch + 1) * 256],
                                kT[kv][:, w0 + ch * 128:w0 + (ch + 1) * 128],
                                qT[p][:, t * 256:(t + 1) * 256],
                                start=True, stop=True)
                        # probs = exp(score)*PSCALE via the ln-bias, straight
                        # to fp8; then zero the masked spans in place
                        probs = probs_pool.tile([128, NCH * 256], f8)
                        nc.scalar.activation(probs[:], scp[:], AF.Exp,
                                             bias=lnps[:], scale=1.0)
                        nc.gpsimd.tensor_mul(probs[:, 0:w1], probs[:, 0:w1],
                                             masks[(blk, qc)][:, 0:w1])
                        nc.gpsimd.tensor_mul(
                            probs[:, 1024:1280], probs[:, 1024:1280],
                            masks[(blk, qc)][:, 1024:1280])
                        if pending is not None:
                            emit_reduce(*pending)
                            if pending[3] == H // 2 - 1:
                                emit_outproj(4 * pending[1] + pending[2])
                        pending = (probs, blk, qc, p)
            emit_reduce(*pending)
            emit_outproj(4 * pending[1] + pending[2])
        qT_cm.__exit__(None, None, None)
        v_cm.__exit__(None, None, None)
        kT_cm.__exit__(None, None, None)

        emb_cm.__exit__(None, None, None)
        wo_cm.__exit__(None, None, None)
        acat_cm.__exit__(None, None, None)


_CACHED_NC = None


def build_graph():
    global _CACHED_NC
    if _CACHED_NC is not None:
        return _CACHED_NC
    nc = bacc.Bacc("TRN2", target_bir_lowering=False, debug=False,
                   enable_asserts=False, num_devices=8)
    d = {}
    d["emb8"] = nc.dram_tensor("emb8", [128, 3, NE, 512], f8,
                               kind="ExternalInput").ap()
    d["ident"] = nc.dram_tensor("ident", [128, 128], bf,
                                kind="ExternalInput").ap()
    d["emb_own"] = nc.dram_tensor("emb_own", [TOWN, E], bf,
                                  kind="ExternalInput").ap()
    d["wq8"] = nc.dram_tensor("wq8", [H, 128, NE, DK], f8,
                              kind="ExternalInput").ap()
    d["wk8"] = nc.dram_tensor("wk8", [128, NE, HKV * DK], f8,
                              kind="ExternalInput").ap()
    d["wv8"] = nc.dram_tensor("wv8", [128, NE, HKV * DV], f8,
                              kind="ExternalInput").ap()
    d["wo8"] = nc.dram_tensor("wo8", [128, H, E], f8,
                              kind="ExternalInput").ap()
    d["cosqT"] = nc.dram_tensor("cosqT", [DK, TOWN], bf, kind="ExternalInput").ap()
    d["sinqT"] = nc.dram_tensor("sinqT", [DK, TOWN], bf, kind="ExternalInput").ap()
    d["coskT"] = nc.dram_tensor("coskT", [DK, TALL], bf, kind="ExternalInput").ap()
    d["sinkT"] = nc.dram_tensor("sinkT", [DK, TALL], bf, kind="ExternalInput").ap()
    d["maskT"] = nc.dram_tensor("maskT", [NB, NQC, 128, NCH, 2, 128], f8,
                                kind="ExternalInput").ap()
    d["out"] = nc.dram_tensor("out", [TOWN, E], bf, kind="ExternalOutput").ap()

    with tile.TileContext(nc, trace_sim=False) as tc:
        build(tc, d)
    nc.compile()
    _CACHED_NC = nc
    return nc


def make_in_maps(embeddings, cos_buffer, sin_buffer, wq, wk, wv, wo):
    embeddings = np.asarray(embeddings, dtype=np.float32)
    cos_buffer = np.asarray(cos_buffer, dtype=np.float32)
    sin_buffer = np.asarray(sin_buffer, dtype=np.float32)
    # [E, H*DK] -> [H, 128, NE, DK] fp8 (k-tile-pair packed, per head).
    # Weights pre-scaled by WSCALE for e4m3 range; the whole 1/sqrt(DK)
    # score scale plus both WSCALE compensations ride on the q-side
    # cos/sin (q) and k-side cos/sin (k) host buffers.
    ws = float(WSCALE)
    wq_s = np.asarray(wq, np.float32) * ws
    wq_s = wq_s.reshape(NE, 128, H, DK).transpose(2, 1, 0, 3)
    wq8 = np.ascontiguousarray(wq_s).astype(E4M3)
    # [E, HKV*DK] -> [128, NE, HKV*DK]
    wk8 = np.ascontiguousarray(
        (np.asarray(wk, np.float32) * ws).reshape(NE, 128, HKV * DK)
        .transpose(1, 0, 2)).astype(E4M3)
    wv8 = np.ascontiguousarray(
        (np.asarray(wv, np.float32) * ws).reshape(NE, 128, HKV * DV)
        .transpose(1, 0, 2)).astype(E4M3)
    # [H*DV, E] -> [128(dv), H, E]
    wo8 = np.ascontiguousarray(
        (np.asarray(wo, np.float32) * ws).reshape(H, DV, E)
        .transpose(1, 0, 2)).astype(E4M3)
    aq = 1.0 / (ws * math.sqrt(DK))   # q-side compensation (+ score scale)
    ak = 1.0 / ws                     # k-side compensation

    in_maps = []
    for core in range(8):
        b, c = divmod(core, 4)
        tok0 = 1024 * c
        if c == 0:
            pad = np.zeros((HALO, E), np.float32)
            seg = np.concatenate([pad, embeddings[b, :TOWN]], axis=0)
            padc = np.zeros((HALO, DK), np.float32)
            ck = np.concatenate([padc, cos_buffer[1, 0, :TOWN]], axis=0)
            sk = np.concatenate([padc, sin_buffer[1, 0, :TOWN]], axis=0)
        else:
            seg = embeddings[b, tok0 - HALO:tok0 + TOWN]
            ck = cos_buffer[1, 0, tok0 - HALO:tok0 + TOWN]
            sk = sin_buffer[1, 0, tok0 - HALO:tok0 + TOWN]

        # [TALL, E] -> [128, 3(tok chunk), NE, 512] fp8
        emb8 = np.ascontiguousarray(
            seg.T.reshape(NE, 128, 3, 512).transpose(1, 2, 0, 3)).astype(E4M3)

        # masks [NB, NQC, 128(kk), NCH, 2(head), 128(qq)] {0,1}
        mask = np.zeros((NB, NQC, 128, NCH, 2, 128), np.float32)
        qq = np.arange(128)
        kk = np.arange(128)
        for blk in range(NB):
            for qc in range(NQC):
                qpos = tok0 + 512 * blk + 128 * qc + qq
                for ch in range(NCH):
                    kpos = tok0 - 512 + 512 * blk + 128 * qc + 128 * ch + kk
                    m = ((kpos[:, None] > qpos[None, :] - WIN)
                         & (kpos[:, None] <= qpos[None, :])
                         & (kpos[:, None] >= 0))
                    mask[blk, qc, :, ch, 0, :] = m
                    mask[blk, qc, :, ch, 1, :] = m

        in_maps.append({
            "emb8": emb8,
            "ident": np.eye(128, dtype=np.float32).astype(BF16),
            "emb_own": np.ascontiguousarray(
                embeddings[b, tok0:tok0 + TOWN]).astype(BF16),
            "wq8": wq8, "wk8": wk8, "wv8": wv8, "wo8": wo8,
            "cosqT": np.ascontiguousarray(
                cos_buffer[0, 0, tok0:tok0 + TOWN].T * aq).astype(BF16),
            "sinqT": np.ascontiguousarray(
                sin_buffer[0, 0, tok0:tok0 + TOWN].T * aq).astype(BF16),
            "coskT": np.ascontiguousarray(ck.T * ak).astype(BF16),
            "sinkT": np.ascontiguousarray(sk.T * ak).astype(BF16),
            "maskT": mask.astype(E4M3),
        })
    return in_maps


def _install_ntff_hook():
    """Recreate the missing antenv.axon_hooks registry so
    run_bass_kernel_spmd(trace=True) can capture an NTFF profile."""
    import types
    if "antenv.axon_hooks" not in sys.modules:
        m = types.ModuleType("antenv.axon_hooks")
        m._hook = None
        m.set_axon_ntff_profile_hook = lambda h: setattr(m, "_hook", h)
        m.get_axon_ntff_profile_hook = lambda: m._hook
        sys.modules["antenv.axon_hooks"] = m
        try:
            import antenv
            antenv.axon_hooks = m
        except ImportError:
            pass
    try:
        from trn_agent_boot.trn_boot import _ntff_profile_via_ctypes
        hook = _ntff_profile_via_ctypes("/opt/axon/libaxon_pjrt.so")
        sys.modules["antenv.axon_hooks"].set_axon_ntff_profile_hook(hook)
    except Exception as exc:  # degrade to no tracing
        print(f"ntff hook install failed: {exc}", file=sys.stderr)


def kernel(embeddings, cos_buffer, sin_buffer, wq, wk, wv, wo, window_size,
           trace=False):
    assert int(window_size) == WIN
    if trace:
        _install_ntff_hook()
    nc = build_graph()
    in_maps = make_in_maps(embeddings, cos_buffer, sin_buffer, wq, wk, wv, wo)
    if trace:
        # warm-up executions: ramp device clocks so the traced run below
        # measures the steady-state rate
        for _ in range(2):
            bass_utils.run_bass_kernel_spmd(
                nc, in_maps, core_ids=list(range(8)), trace=False)
    res = bass_utils.run_bass_kernel_spmd(
        nc, in_maps, core_ids=list(range(8)), trace=trace)
    out = np.zeros((B, S, E), np.float32)
    for core in range(8):
        b, c = divmod(core, 4)
        out[b, 1024 * c:1024 * (c + 1)] = np.asarray(
            res.results[core]["out"]).astype(np.float32)
    if trace:
        kernel.last_exec_time_ns = res.exec_time_ns
    return out


kernel.last_exec_time_ns = None
